# revision 24
# baseline (speedup 1.0000x reference)
"""Cross multi-head attention TRN2 kernel (8-core SPMD, head-sharded).

Strategy (tensor parallel over heads, zero communication):
  - 16 heads / 8 cores -> 2 heads per core. Core c computes output columns
    [128*c, 128*(c+1)) of the [4096, 1024] output; host concatenates.
  - Host pre-transposes q/embed to [E, rows] and casts to bf16 so the
    contraction dim (E) lands on SBUF partitions with no on-chip transposes.
  - Scores are computed transposed (S^T[k, q] = K.Q^T, scale folded into Wq).
    The two heads' K=64-contraction score matmuls are issued back-to-back to
    PE row groups 0/64 (tile_position auto-derived from base partitions) so
    they execute CONCURRENTLY in the systolic array -> scores cost ~halves.
  - One exp activation per key-chunk covers both heads [128, 1024]; ACT is
    the pacing engine (~1.1us per instr, 128 instrs). Softmax skips the
    max-subtraction (logits ~ N(0,1)) and the denominator comes from a
    ones-column appended to V, so attn.V also produces row-sums.
  - Emission order per key-chunk slot is scores(kc) / exp(kc) / fillers /
    ctx(kc-1): the in-order PE never parks on a ctx matmul whose probs
    aren't ready, so the exp stream stays dense.
  - ctx'^T [80, 512] (V padded to 80 cols: 64 d + ones + 15 zeros) is cast
    to bf16 and transposed by the DMA XBAR (not the PE), then normalized
    per-partition on DVE and DMA'd out. PE does only proj+scores+ctx.
  - Projections are emitted as ~2-matmul "filler" pieces on an explicit
    per-slot schedule so the PE uses its slack without stalling ACT.
"""

import numpy as np
import ml_dtypes

import concourse.bass as bass
import concourse.bacc as bacc
import concourse.mybir as mybir
import concourse.tile as tile
from concourse.bass_utils import run_bass_kernel_spmd

# ---- problem dims (hardcoded; kernel.py must be self-contained) ----
B, S, E = 2, 2048, 1024
NHEAD, HD = 16, 64
NCORES = 8
HPC = NHEAD // NCORES          # heads per core = 2
DPC = HPC * HD                 # projection out-dims per core = 128
ROWS = B * S                   # 4096
P = 128                        # SBUF partitions
NFREE = 512                    # matmul moving free dim (one PSUM bank fp32)
EC = E // P                    # 8 contraction chunks
HEC = EC // 2                  # contraction chunks per src-DMA half
KC = S // P                    # 16 key chunks per batch
QC = S // NFREE                # 4 query chunks per batch
RC_B = S // NFREE              # 4 projection row-chunks per batch
VPAD = 80                      # V free dim: 64 d + ones col + 15 zero pad
SCALE = 1.0 / np.sqrt(HD)      # 0.125, folded into Wq/bq on host

F32 = mybir.dt.float32
BF16 = mybir.dt.bfloat16
AF = mybir.ActivationFunctionType

_CACHED_NC = {}
LAST_RESULTS = None            # test.py reads exec_time_ns / profile from here


def _build_nc(with_bias: bool) -> bass.Bass:
    nc = bacc.Bacc(
        "TRN2",
        target_bir_lowering=False,
        debug=False,
        num_devices=NCORES,
    )

    qT = nc.declare_dram_parameter("qT", [E, ROWS], BF16, isOutput=False)
    eT = nc.declare_dram_parameter("eT", [E, ROWS], BF16, isOutput=False)
    WqT = nc.declare_dram_parameter("WqT", [E, DPC], BF16, isOutput=False)
    WkT = nc.declare_dram_parameter("WkT", [E, DPC], BF16, isOutput=False)
    WvT = nc.declare_dram_parameter("WvT", [E, DPC], BF16, isOutput=False)
    bqs = nc.declare_dram_parameter("bqs", [DPC], BF16, isOutput=False)
    bkp = nc.declare_dram_parameter("bkp", [DPC], BF16, isOutput=False)
    bvp = nc.declare_dram_parameter("bvp", [DPC], BF16, isOutput=False)
    out = nc.declare_dram_parameter("out", [ROWS, DPC], F32, isOutput=True)

    with tile.TileContext(nc) as tc:
        with (
            tc.tile_pool(name="consts", bufs=1) as consts,
            tc.tile_pool(name="wpool", bufs=1) as wpool,
            tc.tile_pool(name="resid", bufs=1) as resid,
            tc.tile_pool(name="src", bufs=10) as srcp,
            tc.tile_pool(name="probs", bufs=3) as prp,
            tc.tile_pool(name="ctxb", bufs=4) as cbp,
            tc.tile_pool(name="tpp", bufs=6) as tpp,
            tc.tile_pool(name="rcpp", bufs=4) as rcpp,
            tc.tile_pool(name="otp", bufs=8) as otp,
            tc.tile_pool(name="psmall", bufs=2, space="PSUM") as psmall,
            tc.tile_pool(name="psq", bufs=2, space="PSUM") as psq,
            tc.tile_pool(name="pctx", bufs=2, space="PSUM") as pctx,
        ):
            # ---------- constants & weights ----------
            wq_sb = wpool.tile([P, EC, DPC], BF16)
            wk_sb = wpool.tile([P, EC, DPC], BF16)
            wv_sb = wpool.tile([P, EC, DPC], BF16)

            def dma_weight(w_sb, w_dram):
                nc.sync.dma_start(
                    w_sb, w_dram.ap().rearrange("(c p) d -> p c d", p=P)
                )

            ones_row = consts.tile([1, NFREE], BF16)
            nc.vector.memset(ones_row, 1.0)

            bq_sb = wpool.tile([1, DPC], BF16)
            nc.gpsimd.dma_start(bq_sb, bqs.ap()[None, :])
            bk_sb = wpool.tile([1, DPC], BF16)
            nc.gpsimd.dma_start(bk_sb, bkp.ap()[None, :])
            bv_sb = wpool.tile([1, DPC], BF16)
            nc.gpsimd.dma_start(bv_sb, bvp.ap()[None, :])

            # ---------- residents, chunked to keep dependency tracking
            # (which is tile-granular) from serializing projection writes
            # against attention reads of other chunks ----------
            qt_sb, kt_sb, v_sb = {}, {}, {}
            for b in range(B):
                for r in range(RC_B):
                    qt_sb[b, r] = resid.tile(
                        [P, NFREE], BF16, name=f"qt{b}_{r}"
                    )
                    kt_sb[b, r] = resid.tile(
                        [P, NFREE], BF16, name=f"kt{b}_{r}"
                    )
                for kc in range(KC):
                    vv = resid.tile([P, HPC, VPAD], BF16, name=f"v{b}_{kc}")
                    nc.vector.memset(vv[:, :, HD:VPAD], 0.0)
                    nc.vector.memset(vv[:, :, HD : HD + 1], 1.0)
                    v_sb[b, kc] = vv

            # ---------- projection pieces (fine-grained PE fillers) ----------
            src_tiles = {}

            def do_dma(b, r, key, half=None):
                """DMA one 512-row chunk of q/e, as 2 E-halves."""
                dram = qT if key == "q" else eT
                row0 = b * S + r * NFREE
                for hh in (0, 1) if half is None else (half,):
                    tl = srcp.tile(
                        [P, HEC, NFREE],
                        BF16,
                        tag=f"{key}src",
                        name=f"src{key}{b}_{r}_{hh}",
                    )
                    nc.sync.dma_start(
                        tl,
                        dram.ap()[
                            hh * HEC * P : (hh + 1) * HEC * P,
                            row0 : row0 + NFREE,
                        ].rearrange("(c p) n -> p c n", p=P),
                    )
                    src_tiles.setdefault((b, r, key), []).append(tl)

            def sl(b, r, key, c):
                return src_tiles[(b, r, key)][c // HEC][:, c % HEC]

            _pp = {}

            def qk_piece(b, r, which, part):
                """2 of the 8 accumulating matmuls for a Q/K proj chunk;
                part 3 finishes (optional bias) and casts to qt/kt."""
                w_t = wq_sb if which == "q" else wk_sb
                b_t = bq_sb if which == "q" else bk_sb
                dst = qt_sb[b, r] if which == "q" else kt_sb[b, r]
                skey = "q" if which == "q" else "e"
                if part == 0:
                    _pp[(b, r, which)] = psmall.tile(
                        [P, NFREE], F32, tag="ps", name=f"pp{b}_{r}_{which}"
                    )
                pp = _pp[(b, r, which)]
                for c in range(2 * part, 2 * part + 2):
                    nc.tensor.matmul(
                        pp,
                        lhsT=w_t[:, c],
                        rhs=sl(b, r, skey, c),
                        start=(c == 0),
                        stop=(not with_bias and c == EC - 1),
                    )
                if part == 3:
                    if with_bias:
                        nc.tensor.matmul(
                            pp, lhsT=b_t, rhs=ones_row, start=False, stop=True
                        )
                    nc.vector.tensor_copy(dst, pp)
                    del _pp[(b, r, which)]

            def v_piece(b, kc):
                """V projection for one 128-key chunk (8 small matmuls)."""
                r, sub = kc // 4, kc % 4
                pv = psmall.tile([P, DPC], F32, tag="ps", name=f"pv{b}_{kc}")
                for c in range(EC):
                    nc.tensor.matmul(
                        pv,
                        lhsT=sl(b, r, "e", c)[:, sub * P : (sub + 1) * P],
                        rhs=wv_sb[:, c],
                        start=(c == 0),
                        stop=(not with_bias and c == EC - 1),
                    )
                if with_bias:
                    nc.tensor.matmul(
                        pv,
                        lhsT=ones_row[:, :P],
                        rhs=bv_sb,
                        start=False,
                        stop=True,
                    )
                nc.vector.tensor_copy(
                    v_sb[b, kc][:, :, 0:HD],
                    pv.rearrange("p (h d) -> p h d", h=HPC),
                )

            def QK(b, r, which, part):
                return lambda: qk_piece(b, r, which, part)

            def V(b, kc):
                return lambda: v_piece(b, kc)

            def DMA(b, r, key):
                return lambda: do_dma(b, r, key)

            # ---------- attention ----------
            def attn_iter(b, qc, sched, split=False):
                """One (batch, 512-query chunk): both heads together.
                sched: dict kc -> list of filler closures for that slot.
                split: accumulate ctx in two kc-halves (A: 0..7, B: 8..15)
                so half the evacuation overlaps the kc stream (last iter)."""
                col0 = qc * NFREE
                nhalf = 2 if split else 1
                hk = KC // nhalf
                ctx_ps = [
                    [
                        (pctx if g == 0 or h == 0 else psmall).tile(
                            [VPAD, NFREE],
                            F32,
                            tag="ctx" if g == 0 or h == 0 else "ps",
                            name=f"ctx{b}_{qc}_{h}_{g}",
                        )
                        for h in range(HPC)
                    ]
                    for g in range(nhalf)
                ]
                prs = []
                tps = [[None] * HPC for _ in range(nhalf)]

                def evac_group(g):
                    final = split and g == nhalf - 1
                    for h in range(HPC):
                        cb = cbp.tile(
                            [VPAD, NFREE],
                            BF16,
                            tag="cb",
                            name=f"cb{b}_{qc}_{h}_{g}",
                        )
                        if final and h == 1:
                            nc.scalar.copy(cb, ctx_ps[g][h])
                        else:
                            nc.vector.tensor_copy(cb, ctx_ps[g][h])
                        tp = tpp.tile(
                            [P, NFREE // P, VPAD],
                            BF16,
                            tag="tp",
                            name=f"tp{b}_{qc}_{h}_{g}",
                        )
                        eng = nc.scalar if final and h == 1 else nc.sync
                        eng.dma_start(tp, cb, transpose=True)
                        tps[g][h] = tp

                def emit_ctx(kc):
                    pr = prs[kc]
                    for h in range(HPC):
                        nc.tensor.matmul(
                            ctx_ps[kc // hk][h],
                            lhsT=v_sb[b, kc][:, h, :],
                            rhs=pr[:, h * NFREE : (h + 1) * NFREE],
                            start=(kc % hk == 0),
                            stop=(kc % hk == hk - 1),
                        )

                for kc in range(KC):
                    sp = psq.tile(
                        [P, HPC * NFREE], F32, tag="sps", name=f"sp{b}_{qc}_{kc}"
                    )
                    for h in range(HPC):
                        # h0 -> PE row group 0, h1 -> group 64: concurrent
                        nc.tensor.matmul(
                            sp[:, h * NFREE : (h + 1) * NFREE],
                            lhsT=kt_sb[b, kc // 4][
                                h * HD : (h + 1) * HD,
                                (kc % 4) * P : (kc % 4 + 1) * P,
                            ],
                            rhs=qt_sb[b, qc][h * HD : (h + 1) * HD, :],
                            start=True,
                            stop=True,
                        )
                    pr = prp.tile(
                        [P, HPC * NFREE], BF16, tag="pr", name=f"pr{b}_{qc}_{kc}"
                    )
                    nc.scalar.activation(pr, sp, AF.Exp)
                    prs.append(pr)
                    for f in sched.get(kc, ()):
                        f()
                    if kc >= 1:
                        emit_ctx(kc - 1)
                        if split and kc - 1 == hk - 1:
                            evac_group(0)
                emit_ctx(KC - 1)

                # evacuate: cast -> XBAR transpose -> normalize -> DMA out
                # (group 0 of a split iter was evacuated inside the kc loop)
                evac_group(nhalf - 1)
                for t in range(NFREE // P):
                    ot = otp.tile([P, DPC], F32, tag="ot", name=f"ot{b}_{qc}_{t}")
                    for h in range(HPC):
                        rcp = rcpp.tile(
                            [P, 1], F32, tag="rcp", name=f"rcp{b}_{qc}_{t}_{h}"
                        )
                        if split:
                            tsum = tpp.tile(
                                [P, VPAD],
                                F32,
                                tag="tsum",
                                name=f"tsum{b}_{qc}_{t}_{h}",
                            )
                            eng = nc.gpsimd if h == 0 else nc.vector
                            eng.tensor_add(
                                tsum, tps[0][h][:, t], tps[1][h][:, t]
                            )
                            src_t = tsum
                        else:
                            src_t = tps[0][h][:, t]
                        nc.vector.reciprocal(rcp, src_t[:, HD : HD + 1])
                        nc.vector.tensor_mul(
                            ot[:, h * HD : (h + 1) * HD],
                            src_t[:, 0:HD],
                            rcp.broadcast_to([P, HD]),
                        )
                    row0 = b * S + qc * NFREE + t * P
                    nc.sync.dma_start(out.ap()[row0 : row0 + P, :], ot)

            # ---------- schedule ----------
            # prefix: minimal b0 work so the exp stream starts ASAP.
            # DMAs ordered by first use on the serial sync DMA queue; while
            # the first transfers land, dummy matmuls warm the PE's HAM clock
            # gate (idle default is 1.2 GHz; ~3.4us of activity => 2.4 GHz)
            # so the prefix projections run at full rate.
            dma_weight(wk_sb, WkT)
            do_dma(0, 0, "e", half=0)
            qk_piece(0, 0, "k", 0)
            qk_piece(0, 0, "k", 1)
            do_dma(0, 0, "q", half=0)
            dma_weight(wq_sb, WqT)
            qk_piece(0, 0, "q", 0)
            qk_piece(0, 0, "q", 1)
            do_dma(0, 0, "e", half=1)
            dma_weight(wv_sb, WvT)
            qk_piece(0, 0, "k", 2)
            qk_piece(0, 0, "k", 3)
            do_dma(0, 0, "q", half=1)
            qk_piece(0, 0, "q", 2)
            qk_piece(0, 0, "q", 3)
            do_dma(0, 1, "e")
            do_dma(0, 2, "e")
            do_dma(0, 3, "e")
            do_dma(0, 1, "q")

            # per-(iter, slot) filler schedule; deadlines:
            #   K(b,r) parts by slot 4r-1 of that batch's first iter;
            #   V(b,kc) by slot kc (ctx(kc) is emitted in slot kc+1);
            #   Q(b,r) fully inside iter qc=r-1 (read at next iter's slot 0).
            S_ = {}

            def add(it, kc, *pieces):
                S_.setdefault(it, {}).setdefault(kc, []).extend(pieces)

            # iter 0 = (b0, qc0): finish b0's K/V, then Q(0,1).
            # Exactly 2 pieces per slot; V(0,kc) lands in slot kc (its ctx
            # runs in slot kc+1), K(0,r) parts finish by slot 4r-1.
            for kc in range(KC):
                add(0, kc, V(0, kc))
            for i in range(4):
                add(0, 0 + i, QK(0, 1, "k", i))
                add(0, 4 + i, QK(0, 2, "k", i))
                add(0, 8 + i, QK(0, 3, "k", i))
                add(0, 12 + i, QK(0, 1, "q", i))
            add(0, 15, DMA(0, 2, "q"))

            # iter 1 = (b0, qc1): Q(0,2) + b1 preload begins
            add(1, 0, DMA(1, 0, "e"))
            for i in range(4):
                add(1, 1 + i, QK(0, 2, "q", i))
            add(1, 5, DMA(1, 1, "e"))
            for i in range(4):
                add(1, 6 + i, QK(1, 0, "k", i))
            add(1, 10, V(1, 0))
            add(1, 11, V(1, 1))
            add(1, 12, DMA(0, 3, "q"))
            add(1, 13, V(1, 2))
            add(1, 14, V(1, 3))
            add(1, 15, DMA(1, 2, "e"))

            # iter 2 = (b0, qc2): Q(0,3) + more b1 preload
            for i in range(4):
                add(2, 0 + i, QK(0, 3, "q", i))
            add(2, 4, DMA(1, 0, "q"))
            for i in range(4):
                add(2, 5 + i, QK(1, 1, "k", i))
            add(2, 9, V(1, 4))
            add(2, 10, V(1, 5))
            add(2, 11, V(1, 6))
            add(2, 12, V(1, 7))
            add(2, 13, DMA(1, 3, "e"))

            # iter 3 = (b0, qc3): Q(1,0) + rest of b1 K, some V
            for i in range(4):
                add(3, 0 + i, QK(1, 0, "q", i))
            for i in range(4):
                add(3, 4 + i, QK(1, 2, "k", i))
            for i in range(4):
                add(3, 8 + i, QK(1, 3, "k", i))
            add(3, 12, V(1, 8))
            add(3, 13, V(1, 9))
            add(3, 14, V(1, 10))
            add(3, 15, V(1, 11))

            # iter 4 = (b1, qc0): tail of V(b1) + Q(1,1)
            add(4, 0, V(1, 12))
            add(4, 1, V(1, 13))
            add(4, 2, V(1, 14))
            add(4, 3, V(1, 15))
            add(4, 4, DMA(1, 1, "q"))
            for i in range(4):
                add(4, 5 + i, QK(1, 1, "q", i))
            add(4, 9, DMA(1, 2, "q"))

            # iter 5 = (b1, qc1): Q(1,2)
            for i in range(4):
                add(5, 0 + i, QK(1, 2, "q", i))
            add(5, 4, DMA(1, 3, "q"))

            # iter 6 = (b1, qc2): Q(1,3)
            for i in range(4):
                add(6, 0 + i, QK(1, 3, "q", i))

            it = 0
            for b in range(B):
                for qc in range(QC):
                    attn_iter(b, qc, S_.get(it, {}), split=(it == 7))
                    it += 1

    nc.finalize()
    return nc


def _get_nc(with_bias: bool = True) -> bass.Bass:
    if with_bias not in _CACHED_NC:
        _CACHED_NC[with_bias] = _build_nc(with_bias)
    return _CACHED_NC[with_bias]


def kernel(embed, q, Wk, bk, Wq, bq, Wv, bv, trace=False):
    global LAST_RESULTS
    bf = ml_dtypes.bfloat16
    embed = np.asarray(embed, dtype=np.float32)
    q = np.asarray(q, dtype=np.float32)
    Wk = np.asarray(Wk, dtype=np.float32)
    Wq = np.asarray(Wq, dtype=np.float32)
    Wv = np.asarray(Wv, dtype=np.float32)
    bk = np.asarray(bk, dtype=np.float32)
    bq = np.asarray(bq, dtype=np.float32)
    bv = np.asarray(bv, dtype=np.float32)

    qT = np.ascontiguousarray(q.reshape(ROWS, E).T).astype(bf)
    eT = np.ascontiguousarray(embed.reshape(ROWS, E).T).astype(bf)

    in_maps = []
    for c in range(NCORES):
        slc = slice(c * DPC, (c + 1) * DPC)
        in_maps.append(
            {
                "qT": qT,
                "eT": eT,
                # scores scale folded into Wq/bq (exact: *2^-3)
                "WqT": np.ascontiguousarray((Wq[slc] * SCALE).T).astype(bf),
                "WkT": np.ascontiguousarray(Wk[slc].T).astype(bf),
                "WvT": np.ascontiguousarray(Wv[slc].T).astype(bf),
                "bqs": (bq[slc] * SCALE).astype(bf),
                "bkp": bk[slc].astype(bf),
                "bvp": bv[slc].astype(bf),
            }
        )

    with_bias = bool(bq.any() or bk.any() or bv.any())
    nc = _get_nc(with_bias)
    res = run_bass_kernel_spmd(nc, in_maps, list(range(NCORES)), trace=trace)
    LAST_RESULTS = res

    full = np.empty((ROWS, E), dtype=np.float32)
    for c in range(NCORES):
        full[:, c * DPC : (c + 1) * DPC] = res.results[c]["out"]
    return full.reshape(B, S, E)


# revision 26
# speedup vs baseline: 1.0026x; 1.0026x over previous
"""Cross multi-head attention TRN2 kernel (8-core SPMD, head-sharded).

Strategy (tensor parallel over heads, zero communication):
  - 16 heads / 8 cores -> 2 heads per core. Core c computes output columns
    [128*c, 128*(c+1)) of the [4096, 1024] output; host concatenates.
  - Host pre-transposes q/embed to [E, rows] and casts to bf16 so the
    contraction dim (E) lands on SBUF partitions with no on-chip transposes.
  - Scores are computed transposed (S^T[k, q] = K.Q^T, scale folded into Wq).
    The two heads' K=64-contraction score matmuls are issued back-to-back to
    PE row groups 0/64 (tile_position auto-derived from base partitions) so
    they execute CONCURRENTLY in the systolic array -> scores cost ~halves.
  - One exp activation per key-chunk covers both heads [128, 1024]; ACT is
    the pacing engine (~1.1us per instr, 128 instrs). Softmax skips the
    max-subtraction (logits ~ N(0,1)) and the denominator comes from a
    ones-column appended to V, so attn.V also produces row-sums.
  - Emission order per key-chunk slot is scores(kc) / exp(kc) / fillers /
    ctx(kc-1): the in-order PE never parks on a ctx matmul whose probs
    aren't ready, so the exp stream stays dense.
  - ctx'^T [80, 512] (V padded to 80 cols: 64 d + ones + 15 zeros) is cast
    to bf16 and transposed by the DMA XBAR (not the PE), then normalized
    per-partition on DVE and DMA'd out. PE does only proj+scores+ctx.
  - Projections are emitted as ~2-matmul "filler" pieces on an explicit
    per-slot schedule so the PE uses its slack without stalling ACT.
"""

import numpy as np
import ml_dtypes

import concourse.bass as bass
import concourse.bacc as bacc
import concourse.mybir as mybir
import concourse.tile as tile
from concourse.bass_utils import run_bass_kernel_spmd

# ---- problem dims (hardcoded; kernel.py must be self-contained) ----
B, S, E = 2, 2048, 1024
NHEAD, HD = 16, 64
NCORES = 8
HPC = NHEAD // NCORES          # heads per core = 2
DPC = HPC * HD                 # projection out-dims per core = 128
ROWS = B * S                   # 4096
P = 128                        # SBUF partitions
NFREE = 512                    # matmul moving free dim (one PSUM bank fp32)
EC = E // P                    # 8 contraction chunks
HEC = EC // 2                  # contraction chunks per src-DMA half
KC = S // P                    # 16 key chunks per batch
QC = S // NFREE                # 4 query chunks per batch
RC_B = S // NFREE              # 4 projection row-chunks per batch
VPAD = 80                      # V free dim: 64 d + ones col + 15 zero pad
SCALE = 1.0 / np.sqrt(HD)      # 0.125, folded into Wq/bq on host

F32 = mybir.dt.float32
BF16 = mybir.dt.bfloat16
AF = mybir.ActivationFunctionType

_CACHED_NC = {}
LAST_RESULTS = None            # test.py reads exec_time_ns / profile from here


def _build_nc(with_bias: bool) -> bass.Bass:
    nc = bacc.Bacc(
        "TRN2",
        target_bir_lowering=False,
        debug=False,
        num_devices=NCORES,
    )

    qT = nc.declare_dram_parameter("qT", [E, ROWS], BF16, isOutput=False)
    eT = nc.declare_dram_parameter("eT", [E, ROWS], BF16, isOutput=False)
    WqT = nc.declare_dram_parameter("WqT", [E, DPC], BF16, isOutput=False)
    WkT = nc.declare_dram_parameter("WkT", [E, DPC], BF16, isOutput=False)
    WvT = nc.declare_dram_parameter("WvT", [E, DPC], BF16, isOutput=False)
    bqs = nc.declare_dram_parameter("bqs", [DPC], BF16, isOutput=False)
    bkp = nc.declare_dram_parameter("bkp", [DPC], BF16, isOutput=False)
    bvp = nc.declare_dram_parameter("bvp", [DPC], BF16, isOutput=False)
    out = nc.declare_dram_parameter("out", [ROWS, DPC], F32, isOutput=True)

    with tile.TileContext(nc) as tc:
        with (
            tc.tile_pool(name="consts", bufs=1) as consts,
            tc.tile_pool(name="wpool", bufs=1) as wpool,
            tc.tile_pool(name="resid", bufs=1) as resid,
            tc.tile_pool(name="src", bufs=10) as srcp,
            tc.tile_pool(name="probs", bufs=5) as prp,
            tc.tile_pool(name="ctxb", bufs=4) as cbp,
            tc.tile_pool(name="tpp", bufs=6) as tpp,
            tc.tile_pool(name="rcpp", bufs=4) as rcpp,
            tc.tile_pool(name="otp", bufs=8) as otp,
            tc.tile_pool(name="psmall", bufs=2, space="PSUM") as psmall,
            tc.tile_pool(name="psq", bufs=2, space="PSUM") as psq,
            tc.tile_pool(name="pctx", bufs=2, space="PSUM") as pctx,
        ):
            # ---------- constants & weights ----------
            wq_sb = wpool.tile([P, EC, DPC], BF16)
            wk_sb = wpool.tile([P, EC, DPC], BF16)
            wv_sb = wpool.tile([P, EC, DPC], BF16)

            def dma_weight(w_sb, w_dram):
                nc.sync.dma_start(
                    w_sb, w_dram.ap().rearrange("(c p) d -> p c d", p=P)
                )

            ones_row = consts.tile([1, NFREE], BF16)
            nc.vector.memset(ones_row, 1.0)

            bq_sb = wpool.tile([1, DPC], BF16)
            nc.gpsimd.dma_start(bq_sb, bqs.ap()[None, :])
            bk_sb = wpool.tile([1, DPC], BF16)
            nc.gpsimd.dma_start(bk_sb, bkp.ap()[None, :])
            bv_sb = wpool.tile([1, DPC], BF16)
            nc.gpsimd.dma_start(bv_sb, bvp.ap()[None, :])

            # ---------- residents, chunked to keep dependency tracking
            # (which is tile-granular) from serializing projection writes
            # against attention reads of other chunks ----------
            qt_sb, kt_sb, v_sb = {}, {}, {}
            for b in range(B):
                for r in range(RC_B):
                    qt_sb[b, r] = resid.tile(
                        [P, NFREE], BF16, name=f"qt{b}_{r}"
                    )
                    kt_sb[b, r] = resid.tile(
                        [P, NFREE], BF16, name=f"kt{b}_{r}"
                    )
                for kc in range(KC):
                    vv = resid.tile([P, HPC, VPAD], BF16, name=f"v{b}_{kc}")
                    nc.vector.memset(vv[:, :, HD:VPAD], 0.0)
                    nc.vector.memset(vv[:, :, HD : HD + 1], 1.0)
                    v_sb[b, kc] = vv

            # ---------- projection pieces (fine-grained PE fillers) ----------
            src_tiles = {}

            def do_dma(b, r, key, half=None):
                """DMA one 512-row chunk of q/e, as 2 E-halves."""
                dram = qT if key == "q" else eT
                row0 = b * S + r * NFREE
                for hh in (0, 1) if half is None else (half,):
                    tl = srcp.tile(
                        [P, HEC, NFREE],
                        BF16,
                        tag=f"{key}src",
                        name=f"src{key}{b}_{r}_{hh}",
                    )
                    nc.sync.dma_start(
                        tl,
                        dram.ap()[
                            hh * HEC * P : (hh + 1) * HEC * P,
                            row0 : row0 + NFREE,
                        ].rearrange("(c p) n -> p c n", p=P),
                    )
                    src_tiles.setdefault((b, r, key), []).append(tl)

            def sl(b, r, key, c):
                return src_tiles[(b, r, key)][c // HEC][:, c % HEC]

            _pp = {}

            def qk_piece(b, r, which, part):
                """2 of the 8 accumulating matmuls for a Q/K proj chunk;
                part 3 finishes (optional bias) and casts to qt/kt."""
                w_t = wq_sb if which == "q" else wk_sb
                b_t = bq_sb if which == "q" else bk_sb
                dst = qt_sb[b, r] if which == "q" else kt_sb[b, r]
                skey = "q" if which == "q" else "e"
                if part == 0:
                    _pp[(b, r, which)] = psmall.tile(
                        [P, NFREE], F32, tag="ps", name=f"pp{b}_{r}_{which}"
                    )
                pp = _pp[(b, r, which)]
                for c in range(2 * part, 2 * part + 2):
                    nc.tensor.matmul(
                        pp,
                        lhsT=w_t[:, c],
                        rhs=sl(b, r, skey, c),
                        start=(c == 0),
                        stop=(not with_bias and c == EC - 1),
                    )
                if part == 3:
                    if with_bias:
                        nc.tensor.matmul(
                            pp, lhsT=b_t, rhs=ones_row, start=False, stop=True
                        )
                    nc.vector.tensor_copy(dst, pp)
                    del _pp[(b, r, which)]

            def v_piece(b, kc):
                """V projection for one 128-key chunk (8 small matmuls)."""
                r, sub = kc // 4, kc % 4
                pv = psmall.tile([P, DPC], F32, tag="ps", name=f"pv{b}_{kc}")
                for c in range(EC):
                    nc.tensor.matmul(
                        pv,
                        lhsT=sl(b, r, "e", c)[:, sub * P : (sub + 1) * P],
                        rhs=wv_sb[:, c],
                        start=(c == 0),
                        stop=(not with_bias and c == EC - 1),
                    )
                if with_bias:
                    nc.tensor.matmul(
                        pv,
                        lhsT=ones_row[:, :P],
                        rhs=bv_sb,
                        start=False,
                        stop=True,
                    )
                nc.vector.tensor_copy(
                    v_sb[b, kc][:, :, 0:HD],
                    pv.rearrange("p (h d) -> p h d", h=HPC),
                )

            def QK(b, r, which, part):
                return lambda: qk_piece(b, r, which, part)

            def V(b, kc):
                return lambda: v_piece(b, kc)

            def DMA(b, r, key):
                return lambda: do_dma(b, r, key)

            # ---------- attention ----------
            def attn_iter(b, qc, sched, split=False):
                """One (batch, 512-query chunk): both heads together.
                sched: dict kc -> list of filler closures for that slot.
                split: accumulate ctx in two kc-halves (A: 0..7, B: 8..15)
                so half the evacuation overlaps the kc stream (last iter)."""
                col0 = qc * NFREE
                nhalf = 2 if split else 1
                hk = KC // nhalf
                ctx_ps = [
                    [
                        (pctx if g == 0 or h == 0 else psmall).tile(
                            [VPAD, NFREE],
                            F32,
                            tag="ctx" if g == 0 or h == 0 else "ps",
                            name=f"ctx{b}_{qc}_{h}_{g}",
                        )
                        for h in range(HPC)
                    ]
                    for g in range(nhalf)
                ]
                prs = []
                tps = [[None] * HPC for _ in range(nhalf)]

                def evac_group(g):
                    final = split and g == nhalf - 1
                    for h in range(HPC):
                        cb = cbp.tile(
                            [VPAD, NFREE],
                            BF16,
                            tag="cb",
                            name=f"cb{b}_{qc}_{h}_{g}",
                        )
                        if final and h == 1:
                            nc.scalar.copy(cb, ctx_ps[g][h])
                        else:
                            nc.vector.tensor_copy(cb, ctx_ps[g][h])
                        tp = tpp.tile(
                            [P, NFREE // P, VPAD],
                            BF16,
                            tag="tp",
                            name=f"tp{b}_{qc}_{h}_{g}",
                        )
                        eng = nc.scalar if final and h == 1 else nc.sync
                        eng.dma_start(tp, cb, transpose=True)
                        tps[g][h] = tp

                def emit_ctx(kc):
                    pr = prs[kc]
                    for h in range(HPC):
                        nc.tensor.matmul(
                            ctx_ps[kc // hk][h],
                            lhsT=v_sb[b, kc][:, h, :],
                            rhs=pr[:, h * NFREE : (h + 1) * NFREE],
                            start=(kc % hk == 0),
                            stop=(kc % hk == hk - 1),
                        )

                for kc in range(KC):
                    sp = psq.tile(
                        [P, HPC * NFREE], F32, tag="sps", name=f"sp{b}_{qc}_{kc}"
                    )
                    for h in range(HPC):
                        # h0 -> PE row group 0, h1 -> group 64: concurrent
                        nc.tensor.matmul(
                            sp[:, h * NFREE : (h + 1) * NFREE],
                            lhsT=kt_sb[b, kc // 4][
                                h * HD : (h + 1) * HD,
                                (kc % 4) * P : (kc % 4 + 1) * P,
                            ],
                            rhs=qt_sb[b, qc][h * HD : (h + 1) * HD, :],
                            start=True,
                            stop=True,
                        )
                    pr = prp.tile(
                        [P, HPC * NFREE], BF16, tag="pr", name=f"pr{b}_{qc}_{kc}"
                    )
                    nc.scalar.activation(pr, sp, AF.Exp)
                    prs.append(pr)
                    for f in sched.get(kc, ()):
                        f()
                    if kc >= 1:
                        emit_ctx(kc - 1)
                        if split and kc - 1 == hk - 1:
                            evac_group(0)
                emit_ctx(KC - 1)

                # evacuate: cast -> XBAR transpose -> normalize -> DMA out
                # (group 0 of a split iter was evacuated inside the kc loop)
                evac_group(nhalf - 1)
                for t in range(NFREE // P):
                    ot = otp.tile([P, DPC], F32, tag="ot", name=f"ot{b}_{qc}_{t}")
                    for h in range(HPC):
                        rcp = rcpp.tile(
                            [P, 1], F32, tag="rcp", name=f"rcp{b}_{qc}_{t}_{h}"
                        )
                        if split:
                            tsum = tpp.tile(
                                [P, VPAD],
                                F32,
                                tag="tsum",
                                name=f"tsum{b}_{qc}_{t}_{h}",
                            )
                            eng = nc.gpsimd if h == 0 else nc.vector
                            eng.tensor_add(
                                tsum, tps[0][h][:, t], tps[1][h][:, t]
                            )
                            src_t = tsum
                        else:
                            src_t = tps[0][h][:, t]
                        nc.vector.reciprocal(rcp, src_t[:, HD : HD + 1])
                        nc.vector.tensor_mul(
                            ot[:, h * HD : (h + 1) * HD],
                            src_t[:, 0:HD],
                            rcp.broadcast_to([P, HD]),
                        )
                    row0 = b * S + qc * NFREE + t * P
                    nc.sync.dma_start(out.ap()[row0 : row0 + P, :], ot)

            # ---------- schedule ----------
            # prefix: minimal b0 work so the exp stream starts ASAP.
            # DMAs ordered by first use on the serial sync DMA queue; while
            # the first transfers land, dummy matmuls warm the PE's HAM clock
            # gate (idle default is 1.2 GHz; ~3.4us of activity => 2.4 GHz)
            # so the prefix projections run at full rate.
            dma_weight(wk_sb, WkT)
            do_dma(0, 0, "e", half=0)
            qk_piece(0, 0, "k", 0)
            qk_piece(0, 0, "k", 1)
            do_dma(0, 0, "q", half=0)
            dma_weight(wq_sb, WqT)
            qk_piece(0, 0, "q", 0)
            qk_piece(0, 0, "q", 1)
            do_dma(0, 0, "e", half=1)
            dma_weight(wv_sb, WvT)
            qk_piece(0, 0, "k", 2)
            qk_piece(0, 0, "k", 3)
            do_dma(0, 0, "q", half=1)
            qk_piece(0, 0, "q", 2)
            qk_piece(0, 0, "q", 3)
            do_dma(0, 1, "e")
            do_dma(0, 2, "e")
            do_dma(0, 3, "e")
            do_dma(0, 1, "q")

            # per-(iter, slot) filler schedule; deadlines:
            #   K(b,r) parts by slot 4r-1 of that batch's first iter;
            #   V(b,kc) by slot kc (ctx(kc) is emitted in slot kc+1);
            #   Q(b,r) fully inside iter qc=r-1 (read at next iter's slot 0).
            S_ = {}

            def add(it, kc, *pieces):
                S_.setdefault(it, {}).setdefault(kc, []).extend(pieces)

            # iter 0 = (b0, qc0): finish b0's K/V, then Q(0,1)
            for kc in range(5):
                add(0, kc, V(0, kc))
            for i in range(3):
                add(0, 0 + i, QK(0, 1, "k", i), V(0, 5 + i))
                add(0, 4 + i, QK(0, 2, "k", i), V(0, 8 + i))
                add(0, 8 + i, QK(0, 3, "k", i), V(0, 11 + i))
            add(0, 3, QK(0, 1, "k", 3))
            add(0, 7, QK(0, 2, "k", 3))
            add(0, 11, QK(0, 3, "k", 3))
            add(0, 12, V(0, 14), QK(0, 1, "q", 0))
            add(0, 13, V(0, 15), QK(0, 1, "q", 1))
            add(0, 14, DMA(0, 2, "q"), QK(0, 1, "q", 2))
            add(0, 15, QK(0, 1, "q", 3))

            # iter 1 = (b0, qc1): Q(0,2) + b1 preload begins
            add(1, 0, DMA(1, 0, "e"))
            for i in range(4):
                add(1, 1 + i, QK(0, 2, "q", i))
            add(1, 5, DMA(1, 1, "e"))
            for i in range(4):
                add(1, 6 + i, QK(1, 0, "k", i))
            add(1, 10, V(1, 0))
            add(1, 11, V(1, 1))
            add(1, 12, DMA(0, 3, "q"))
            add(1, 13, V(1, 2))
            add(1, 14, V(1, 3))
            add(1, 15, DMA(1, 2, "e"))

            # iter 2 = (b0, qc2): Q(0,3) + more b1 preload
            for i in range(4):
                add(2, 0 + i, QK(0, 3, "q", i))
            add(2, 4, DMA(1, 0, "q"))
            for i in range(4):
                add(2, 5 + i, QK(1, 1, "k", i))
            add(2, 9, V(1, 4))
            add(2, 10, V(1, 5))
            add(2, 11, V(1, 6))
            add(2, 12, V(1, 7))
            add(2, 13, DMA(1, 3, "e"))

            # iter 3 = (b0, qc3): Q(1,0) + rest of b1 K, some V
            for i in range(4):
                add(3, 0 + i, QK(1, 0, "q", i))
            for i in range(4):
                add(3, 4 + i, QK(1, 2, "k", i))
            for i in range(4):
                add(3, 8 + i, QK(1, 3, "k", i))
            add(3, 12, V(1, 8))
            add(3, 13, V(1, 9))
            add(3, 14, V(1, 10))
            add(3, 15, V(1, 11))

            # iter 4 = (b1, qc0): tail of V(b1) + Q(1,1)
            add(4, 0, V(1, 12))
            add(4, 1, V(1, 13))
            add(4, 2, V(1, 14))
            add(4, 3, V(1, 15))
            add(4, 4, DMA(1, 1, "q"))
            for i in range(4):
                add(4, 5 + i, QK(1, 1, "q", i))
            add(4, 9, DMA(1, 2, "q"))

            # iter 5 = (b1, qc1): Q(1,2)
            for i in range(4):
                add(5, 0 + i, QK(1, 2, "q", i))
            add(5, 4, DMA(1, 3, "q"))

            # iter 6 = (b1, qc2): Q(1,3)
            for i in range(4):
                add(6, 0 + i, QK(1, 3, "q", i))

            it = 0
            for b in range(B):
                for qc in range(QC):
                    attn_iter(b, qc, S_.get(it, {}), split=(it == 7))
                    it += 1

    nc.finalize()
    return nc


def _get_nc(with_bias: bool = True) -> bass.Bass:
    if with_bias not in _CACHED_NC:
        _CACHED_NC[with_bias] = _build_nc(with_bias)
    return _CACHED_NC[with_bias]


def kernel(embed, q, Wk, bk, Wq, bq, Wv, bv, trace=False):
    global LAST_RESULTS
    bf = ml_dtypes.bfloat16
    embed = np.asarray(embed, dtype=np.float32)
    q = np.asarray(q, dtype=np.float32)
    Wk = np.asarray(Wk, dtype=np.float32)
    Wq = np.asarray(Wq, dtype=np.float32)
    Wv = np.asarray(Wv, dtype=np.float32)
    bk = np.asarray(bk, dtype=np.float32)
    bq = np.asarray(bq, dtype=np.float32)
    bv = np.asarray(bv, dtype=np.float32)

    qT = np.ascontiguousarray(q.reshape(ROWS, E).T).astype(bf)
    eT = np.ascontiguousarray(embed.reshape(ROWS, E).T).astype(bf)

    in_maps = []
    for c in range(NCORES):
        slc = slice(c * DPC, (c + 1) * DPC)
        in_maps.append(
            {
                "qT": qT,
                "eT": eT,
                # scores scale folded into Wq/bq (exact: *2^-3)
                "WqT": np.ascontiguousarray((Wq[slc] * SCALE).T).astype(bf),
                "WkT": np.ascontiguousarray(Wk[slc].T).astype(bf),
                "WvT": np.ascontiguousarray(Wv[slc].T).astype(bf),
                "bqs": (bq[slc] * SCALE).astype(bf),
                "bkp": bk[slc].astype(bf),
                "bvp": bv[slc].astype(bf),
            }
        )

    with_bias = bool(bq.any() or bk.any() or bv.any())
    nc = _get_nc(with_bias)
    res = run_bass_kernel_spmd(nc, in_maps, list(range(NCORES)), trace=trace)
    LAST_RESULTS = res

    full = np.empty((ROWS, E), dtype=np.float32)
    for c in range(NCORES):
        full[:, c * DPC : (c + 1) * DPC] = res.results[c]["out"]
    return full.reshape(B, S, E)
